# revision 40
# baseline (speedup 1.0000x reference)
"""Trainium2 kernel for the sobel-perception CNN cell.

Computation (per pixel, circular 3x3 stencil):
    perc = [sobel_x * x, sobel_y * x, x]            # 48 channels
    hidden = relu(W1 @ perc + b1)                   # 128 channels
    out    = W2 @ hidden + b2                       # 16 channels

The depthwise sobel convs share one 2d kernel across channels, so they
commute with the 1x1 channel-mixing conv: folding them into W1 gives
hidden = relu(sum_{dy,dx} M[dy,dx] @ x_shift(dy,dx) + b1).

Device layout (v4):
  * Window per output-row-pair: partitions [dxv(2) x dr(4) x ch(16)] = 128,
    dy folded into the partition stack; the two dxv blocks hold the row
    data at column shifts 0 / +1.
  * mm_a: one K=128 matmul covers dx = -1 (dxv0) and dx = 0 (dxv1).
  * mm_b: dx = +1, K=64.  Emitted in row-disjoint pairs — chunk A reads
    the dxv0 block at free offset +2 (partitions 0-63), chunk B reads the
    dxv1 block at offset +1 (partitions 64-127) — so consecutive mm_b's
    execute concurrently in the PE array (disjoint row groups).
  * mm2 (M=16) packs 4 chunks into one PSUM bank via 4x column tiling
    (tile_position cols 0/32/64/96) -> all 4 copied out per [128,512] op.
  * bf16 operands (PSUM stays f32); relu+bias PSUM->SBUF split between
    Scalar (activation) and Vector (tensor_scalar) engines.
  * One batched 256KB output DMA per group; host reassembles the strips.

Sharding: rows of the 1024x1024 grid split across 8 cores (128 rows each);
the host bakes the circular halos into each core's window slab, so the
device kernel needs no collectives.
"""

import sys

sys.path.insert(0, "/opt/trn_rl_repo")

import ml_dtypes
import numpy as np

import concourse.bass as bass
import concourse.mybir as mybir
from concourse.bass_utils import run_bass_kernel_spmd
from concourse.tile import TileContext

H, W, C, HID = 1024, 1024, 16, 128
NCORES = 8
RPC = H // NCORES  # rows per core
NG = RPC // 2  # groups per core (2 output rows each)
WP = W + 2  # window free length
CH = 512  # matmul free-dim chunk (one PSUM bank of fp32)

_SOBEL_X = np.array([[-1.0, 0.0, 1.0], [-2.0, 0.0, 2.0], [-1.0, 0.0, 1.0]], np.float32)
_SOBEL_Y = np.array([[-1.0, -2.0, -1.0], [0.0, 0.0, 0.0], [1.0, 2.0, 1.0]], np.float32)

F32 = mybir.dt.float32
BF16 = mybir.dt.bfloat16
NPBF16 = ml_dtypes.bfloat16


def build_a_mats(W1: np.ndarray) -> np.ndarray:
    """A[dx][o, dy*16+ch] for dx in (-1, 0, +1) -> shape (3, 128, 48)."""
    W1a, W1b, W1c = W1[:, 0:C], W1[:, C : 2 * C], W1[:, 2 * C : 3 * C]
    A = np.zeros((3, HID, 3 * C), np.float32)
    for dxi in range(3):
        for dyi in range(3):
            m = _SOBEL_X[dyi, dxi] * W1a + _SOBEL_Y[dyi, dxi] * W1b
            if dyi == 1 and dxi == 1:
                m = m + W1c
            A[dxi, :, dyi * C : (dyi + 1) * C] = m
    return A


def build_wt(W1: np.ndarray) -> np.ndarray:
    """lhsT slab [128, 3*128] for the all-K=64 layout: row-half i holds the
    three dx matrices for output row i:
    wt[64i + dr*16 + ch, dxi*128 + o] = M[dy=dr-1-i, dx=dxi-1][o, ch]
    (zero outside 0 <= dr-i <= 2)."""
    A = build_a_mats(W1)
    wt = np.zeros((128, 3 * HID), np.float32)
    for i in range(2):
        for dxi in range(3):
            for dr in range(4):
                dyi = dr - i
                if 0 <= dyi <= 2:
                    blk = A[dxi][:, dyi * C : (dyi + 1) * C]  # (128, 16)
                    p0 = 64 * i + dr * C
                    wt[p0 : p0 + C, dxi * HID : (dxi + 1) * HID] = blk.T
    return wt


def _hoist_matmul_waits(nc: bass.Bass) -> None:
    """This walrus build's instruction formats hold at most ONE sync wait,
    but Tile emits 2-3 on some instructions.  Hoist excess waits onto
    inserted same-engine NoOps (one wait each) right before the
    instruction — semantically the same blocking point on the in-order
    engine queue."""
    fixn = 0
    for fn in nc.m.functions:
        for blk in fn.blocks:
            needs_fix = any(
                inst.sync_info is not None and len(inst.sync_info.on_wait) > 1
                for inst in blk.instructions
            )
            if not needs_fix:
                continue
            out = []
            for inst in blk.instructions:
                si = inst.sync_info
                if si is not None and len(si.on_wait) > 1:
                    for w in si.on_wait:
                        nop = mybir.InstNoOp(name=f"I-mmfix-{fixn}")
                        fixn += 1
                        nop.engine = inst.engine
                        nop.sync_info = mybir.SyncInfo(on_wait=[w], on_update=[])
                        out.append(nop)
                    si.on_wait = []
                out.append(inst)
            blk.instructions = out


def build_nc(hoist: bool = True) -> bass.Bass:
    nc = bass.Bass()
    xw = nc.declare_dram_parameter("xw", [NG, 128, WP], BF16, isOutput=False)
    wt = nc.declare_dram_parameter("wt", [128, 3 * HID], BF16, isOutput=False)
    w2t = nc.declare_dram_parameter("w2t", [HID, C], BF16, isOutput=False)
    b1 = nc.declare_dram_parameter("b1", [HID, 1], F32, isOutput=False)
    # raw [gpair][strip j = 2i+h][g%2 half][c] layout; host reassembles
    out = nc.declare_dram_parameter("out", [NG // 2, 128, 2 * CH], BF16, isOutput=True)

    with TileContext(nc) as tc:
        with (
            tc.tile_pool(name="const", bufs=1) as cpool,
            tc.tile_pool(name="xrows", bufs=8) as xpool,
            tc.tile_pool(name="hid", bufs=8) as hpool,
            tc.tile_pool(name="stage", bufs=6) as spool,
            tc.tile_pool(name="cps", bufs=3, space="PSUM") as cps,
            tc.tile_pool(name="ops", bufs=1, space="PSUM") as ops,
        ):
            # constants ride the scalar engine's HWDGE ring so the sync
            # ring streams the first window slabs without queueing behind them
            wt_t = cpool.tile([128, 3 * HID], BF16)
            nc.scalar.dma_start(out=wt_t[:], in_=wt[:])
            w2t_t = cpool.tile([HID, C], BF16)
            nc.scalar.dma_start(out=w2t_t[:], in_=w2t[:])
            b1_t = cpool.tile([HID, 1], F32)
            nc.scalar.dma_start(out=b1_t[:], in_=b1[:])

            wins: dict = {}
            cvs: dict = {}
            ots: dict = {0: None}

            def emit_dma(g):
                win = xpool.tile([128, WP], BF16, tag="xrow", name=f"xw{g}")
                nc.sync.dma_start(out=win[:], in_=xw[g, :, :])
                wins[g] = win

            def emit_mm1(g):
                # all-K=64 mm1: 3 accumulating dx matmuls per chunk; the two
                # output rows (i) live in disjoint row-halves, so adjacent
                # i0/i1 matmuls run concurrently and every weight load hides
                # under the other row-half's stream
                win = wins.pop(g)
                # cv tile per h (col-halves = i): the h0 tile's chains finish
                # at the midpoint of this group's matmul burst, so its relu
                # starts early and frees the buffer for the next group in time
                cv = [
                    cps.tile([128, 2 * CH], F32, tag="cv", name=f"cv{g}_{h}")
                    for h in range(2)
                ]
                for h in range(2):
                    for dxi in range(3):
                        for i in range(2):
                            nc.tensor.matmul(
                                cv[h][:, i * CH : (i + 1) * CH],
                                wt_t[64 * i : 64 * i + 64, dxi * HID : (dxi + 1) * HID],
                                win[64 * i : 64 * i + 64, h * CH + dxi : h * CH + dxi + CH],
                                start=(dxi == 0),
                                stop=(dxi == 2),
                                tile_position=(64 * i, 0),
                            )
                cvs[g] = cv

            def emit_drain(g):
                cv = cvs.pop(g)
                # relu h0 on ACT (faster, tile ready earliest), h1 on DVE
                hid = []
                for h in range(2):
                    ht = hpool.tile([128, 2 * CH], BF16, tag="h", name=f"h{g}_{h}")
                    if h == 0:
                        nc.scalar.activation(
                            ht[:],
                            cv[h][:],
                            mybir.ActivationFunctionType.Relu,
                            bias=b1_t[:],
                            scale=1.0,
                        )
                    else:
                        nc.vector.tensor_scalar(
                            ht[:],
                            cv[h][:],
                            b1_t[:],
                            0.0,
                            mybir.AluOpType.add,
                            mybir.AluOpType.max,
                        )
                    hid.append(ht)

                # mm2: 4 col-tiled strips, free-half g%2 of a 2-bank ops
                # tile shared by a group PAIR; one [128,1024] copy + one
                # 256KB DMA drains both groups
                if g % 2 == 0:
                    ots[0] = ops.tile([128, 2 * CH], F32, tag="o", name=f"o{g}")
                ot = ots[0]
                c0 = (g % 2) * CH
                for h in range(2):
                    for i in range(2):
                        j = 2 * i + h
                        nc.tensor.matmul(
                            ot[32 * j : 32 * j + C, c0 : c0 + CH],
                            w2t_t[:],
                            hid[h][:, i * CH : (i + 1) * CH],
                            start=True,
                            stop=True,
                            tile_position=(0, 32 * j),
                        )
                if g % 2 == 1:
                    st = spool.tile([128, 2 * CH], BF16, tag="st", name=f"st{g}")
                    if (g // 2) % 5 < 3:
                        nc.scalar.activation(
                            st[:], ot[:], mybir.ActivationFunctionType.Copy,
                            bias=0.0, scale=1.0,
                        )
                    else:
                        nc.vector.tensor_copy(st[:], ot[:])
                    nc.gpsimd.dma_start(out=out[g // 2, :, :], in_=st[:])

            # software-pipelined emission: mm1 runs one group ahead of the
            # drain stage so the in-order PE queue never parks on a
            # relu-gated mm2 while the next group's mm1 is ready; window
            # DMAs prefetch four groups ahead
            for g0 in range(4):
                emit_dma(g0)
            emit_mm1(0)
            for g in range(NG):
                if g + 4 < NG:
                    emit_dma(g + 4)
                if g + 1 < NG:
                    emit_mm1(g + 1)
                emit_drain(g)

    if hoist:
        _hoist_matmul_waits(nc)
    return nc


_NC_CACHE: dict = {}


def _get_nc():
    if "nc" not in _NC_CACHE:
        _NC_CACHE["nc"] = build_nc()
    return _NC_CACHE["nc"]


def host_prepare(state, W1, b1, W2):
    """Build per-core input maps. state: (H, W, C) f32."""
    xt = np.ascontiguousarray(state.transpose(2, 0, 1))  # (C, H, W)
    xtp = np.pad(xt, ((0, 0), (1, 1), (1, 1)), mode="wrap")  # (C, H+2, W+2)
    xtp_bf = xtp.astype(NPBF16)
    wtm = build_wt(W1).astype(NPBF16)  # (128, 384)
    w2t = np.ascontiguousarray(W2.T).astype(NPBF16)  # (128, 16)
    b1c = np.ascontiguousarray(b1.reshape(HID, 1)).astype(np.float32)

    in_maps = []
    for k in range(NCORES):
        r0 = k * RPC
        # xw[g, 64i + dr*16 + ch, q] = xtp[ch, r0 + 2g + dr, q]  (both halves)
        slab = np.empty((NG, 128, WP), NPBF16)
        for dr in range(4):
            rows = xtp_bf[:, r0 + dr : r0 + dr + 2 * NG : 2, :].transpose(1, 0, 2)
            slab[:, dr * C : dr * C + C] = rows
            slab[:, 64 + dr * C : 64 + dr * C + C] = rows
        in_maps.append(
            {
                "xw": np.ascontiguousarray(slab),
                "wt": wtm,
                "w2t": w2t,
                "b1": b1c,
            }
        )
    return in_maps


def assemble_out(results, b2):
    """results[k]["out"]: (NG, 128, CH) raw strips -> (H, W, C) + b2."""
    cores = []
    for k in range(NCORES):
        raw = np.asarray(results[k]["out"]).astype(np.float32)  # (NG/2,128,2CH)
        v = raw.reshape(NG // 2, 2, 2, 32, 2, CH)[:, :, :, :C, :, :]
        # dims: (pair k, i, h, ch, half, c); row = 4k + 2*half + i
        core = v.transpose(0, 4, 1, 3, 2, 5).reshape(RPC, C, W)  # row,ch,col
        cores.append(core)
    out_t = np.concatenate(cores, axis=0)  # (H, C, W)
    return np.ascontiguousarray(
        out_t.transpose(0, 2, 1) + b2[None, None, :]
    ).astype(np.float32)


def kernel(state, W1, b1, W2, b2, **extra):
    state = np.asarray(state, np.float32)
    W1 = np.asarray(W1, np.float32)
    b1 = np.asarray(b1, np.float32)
    W2 = np.asarray(W2, np.float32)
    b2 = np.asarray(b2, np.float32)

    nc = _get_nc()
    in_maps = host_prepare(state, W1, b1, W2)
    res = run_bass_kernel_spmd(nc, in_maps, core_ids=list(range(NCORES)))
    return assemble_out(res.results, b2)


if __name__ == "__main__":
    rng = np.random.default_rng(0)
    state = rng.standard_normal((H, W, C), dtype=np.float32)
    W1 = rng.standard_normal((HID, 3 * C), dtype=np.float32) * 0.1
    b1v = rng.standard_normal(HID).astype(np.float32) * 0.1
    W2 = rng.standard_normal((C, HID), dtype=np.float32) * 0.1
    b2v = rng.standard_normal(C).astype(np.float32) * 0.1
    out = kernel(state, W1, b1v, W2, b2v)
    print(out.shape, out.dtype)


# revision 41
# speedup vs baseline: 1.0879x; 1.0879x over previous
"""Trainium2 kernel for the sobel-perception CNN cell.

Computation (per pixel, circular 3x3 stencil):
    perc = [sobel_x * x, sobel_y * x, x]            # 48 channels
    hidden = relu(W1 @ perc + b1)                   # 128 channels
    out    = W2 @ hidden + b2                       # 16 channels

The depthwise sobel convs share one 2d kernel across channels, so they
commute with the 1x1 channel-mixing conv: folding them into W1 gives
hidden = relu(sum_{dy,dx} M[dy,dx] @ x_shift(dy,dx) + b1).

Device layout (v4):
  * Window per output-row-pair: partitions [dxv(2) x dr(4) x ch(16)] = 128,
    dy folded into the partition stack; the two dxv blocks hold the row
    data at column shifts 0 / +1.
  * mm_a: one K=128 matmul covers dx = -1 (dxv0) and dx = 0 (dxv1).
  * mm_b: dx = +1, K=64.  Emitted in row-disjoint pairs — chunk A reads
    the dxv0 block at free offset +2 (partitions 0-63), chunk B reads the
    dxv1 block at offset +1 (partitions 64-127) — so consecutive mm_b's
    execute concurrently in the PE array (disjoint row groups).
  * mm2 (M=16) packs 4 chunks into one PSUM bank via 4x column tiling
    (tile_position cols 0/32/64/96) -> all 4 copied out per [128,512] op.
  * bf16 operands (PSUM stays f32); relu+bias PSUM->SBUF split between
    Scalar (activation) and Vector (tensor_scalar) engines.
  * One batched 256KB output DMA per group; host reassembles the strips.

Sharding: rows of the 1024x1024 grid split across 8 cores (128 rows each);
the host bakes the circular halos into each core's window slab, so the
device kernel needs no collectives.
"""

import sys

sys.path.insert(0, "/opt/trn_rl_repo")

import ml_dtypes
import numpy as np

import concourse.bass as bass
import concourse.mybir as mybir
from concourse.bass_utils import run_bass_kernel_spmd
from concourse.tile import TileContext

H, W, C, HID = 1024, 1024, 16, 128
NCORES = 8
RPC = H // NCORES  # rows per core
NG = RPC // 2  # groups per core (2 output rows each)
WP = W + 2  # window free length
CH = 512  # matmul free-dim chunk (one PSUM bank of fp32)

_SOBEL_X = np.array([[-1.0, 0.0, 1.0], [-2.0, 0.0, 2.0], [-1.0, 0.0, 1.0]], np.float32)
_SOBEL_Y = np.array([[-1.0, -2.0, -1.0], [0.0, 0.0, 0.0], [1.0, 2.0, 1.0]], np.float32)

F32 = mybir.dt.float32
BF16 = mybir.dt.bfloat16
NPBF16 = ml_dtypes.bfloat16


def build_a_mats(W1: np.ndarray) -> np.ndarray:
    """A[dx][o, dy*16+ch] for dx in (-1, 0, +1) -> shape (3, 128, 48)."""
    W1a, W1b, W1c = W1[:, 0:C], W1[:, C : 2 * C], W1[:, 2 * C : 3 * C]
    A = np.zeros((3, HID, 3 * C), np.float32)
    for dxi in range(3):
        for dyi in range(3):
            m = _SOBEL_X[dyi, dxi] * W1a + _SOBEL_Y[dyi, dxi] * W1b
            if dyi == 1 and dxi == 1:
                m = m + W1c
            A[dxi, :, dyi * C : (dyi + 1) * C] = m
    return A


def build_wt(W1: np.ndarray) -> np.ndarray:
    """lhsT slab [128, 3*128] for the all-K=64 layout: row-half i holds the
    three dx matrices for output row i:
    wt[64i + dr*16 + ch, dxi*128 + o] = M[dy=dr-1-i, dx=dxi-1][o, ch]
    (zero outside 0 <= dr-i <= 2)."""
    A = build_a_mats(W1)
    wt = np.zeros((128, 3 * HID), np.float32)
    for i in range(2):
        for dxi in range(3):
            for dr in range(4):
                dyi = dr - i
                if 0 <= dyi <= 2:
                    blk = A[dxi][:, dyi * C : (dyi + 1) * C]  # (128, 16)
                    p0 = 64 * i + dr * C
                    wt[p0 : p0 + C, dxi * HID : (dxi + 1) * HID] = blk.T
    return wt


def _hoist_matmul_waits(nc: bass.Bass) -> None:
    """This walrus build's instruction formats hold at most ONE sync wait,
    but Tile emits 2-3 on some instructions.  Hoist excess waits onto
    inserted same-engine NoOps (one wait each) right before the
    instruction — semantically the same blocking point on the in-order
    engine queue."""
    fixn = 0
    for fn in nc.m.functions:
        for blk in fn.blocks:
            needs_fix = any(
                inst.sync_info is not None and len(inst.sync_info.on_wait) > 1
                for inst in blk.instructions
            )
            if not needs_fix:
                continue
            out = []
            for inst in blk.instructions:
                si = inst.sync_info
                if si is not None and len(si.on_wait) > 1:
                    for w in si.on_wait:
                        nop = mybir.InstNoOp(name=f"I-mmfix-{fixn}")
                        fixn += 1
                        nop.engine = inst.engine
                        nop.sync_info = mybir.SyncInfo(on_wait=[w], on_update=[])
                        out.append(nop)
                    si.on_wait = []
                out.append(inst)
            blk.instructions = out


def build_nc(hoist: bool = True) -> bass.Bass:
    nc = bass.Bass()
    xw = nc.declare_dram_parameter("xw", [NG, 128, WP], BF16, isOutput=False)
    wt = nc.declare_dram_parameter("wt", [128, 3 * HID], BF16, isOutput=False)
    w2t = nc.declare_dram_parameter("w2t", [HID, C], BF16, isOutput=False)
    b1 = nc.declare_dram_parameter("b1", [HID, 1], F32, isOutput=False)
    # raw [g][strip j = 2i+h][ch][c] layout; host reassembles rows/cols
    out = nc.declare_dram_parameter("out", [NG, 128, CH], BF16, isOutput=True)

    with TileContext(nc) as tc:
        with (
            tc.tile_pool(name="const", bufs=1) as cpool,
            tc.tile_pool(name="xrows", bufs=8) as xpool,
            tc.tile_pool(name="hid", bufs=8) as hpool,
            tc.tile_pool(name="stage", bufs=6) as spool,
            tc.tile_pool(name="cps", bufs=3, space="PSUM") as cps,
            tc.tile_pool(name="ops", bufs=2, space="PSUM") as ops,
        ):
            wt_t = cpool.tile([128, 3 * HID], BF16)
            nc.sync.dma_start(out=wt_t[:], in_=wt[:])
            w2t_t = cpool.tile([HID, C], BF16)
            nc.sync.dma_start(out=w2t_t[:], in_=w2t[:])
            b1_t = cpool.tile([HID, 1], F32)
            nc.sync.dma_start(out=b1_t[:], in_=b1[:])

            wins: dict = {}
            cvs: dict = {}

            def emit_dma(g):
                win = xpool.tile([128, WP], BF16, tag="xrow", name=f"xw{g}")
                nc.sync.dma_start(out=win[:], in_=xw[g, :, :])
                wins[g] = win

            def emit_mm1(g):
                # all-K=64 mm1: 3 accumulating dx matmuls per chunk; the two
                # output rows (i) live in disjoint row-halves, so adjacent
                # i0/i1 matmuls run concurrently and every weight load hides
                # under the other row-half's stream
                win = wins.pop(g)
                # cv tile per h (col-halves = i): the h0 tile's chains finish
                # at the midpoint of this group's matmul burst, so its relu
                # starts early and frees the buffer for the next group in time
                cv = [
                    cps.tile([128, 2 * CH], F32, tag="cv", name=f"cv{g}_{h}")
                    for h in range(2)
                ]
                for h in range(2):
                    for dxi in range(3):
                        for i in range(2):
                            nc.tensor.matmul(
                                cv[h][:, i * CH : (i + 1) * CH],
                                wt_t[64 * i : 64 * i + 64, dxi * HID : (dxi + 1) * HID],
                                win[64 * i : 64 * i + 64, h * CH + dxi : h * CH + dxi + CH],
                                start=(dxi == 0),
                                stop=(dxi == 2),
                                tile_position=(64 * i, 0),
                            )
                cvs[g] = cv

            def emit_drain(g):
                cv = cvs.pop(g)
                # relu h0 on ACT (faster, tile ready earliest), h1 on DVE
                hid = []
                for h in range(2):
                    ht = hpool.tile([128, 2 * CH], BF16, tag="h", name=f"h{g}_{h}")
                    if h == 0:
                        nc.scalar.activation(
                            ht[:],
                            cv[h][:],
                            mybir.ActivationFunctionType.Relu,
                            bias=b1_t[:],
                            scale=1.0,
                        )
                    else:
                        nc.vector.tensor_scalar(
                            ht[:],
                            cv[h][:],
                            b1_t[:],
                            0.0,
                            mybir.AluOpType.add,
                            mybir.AluOpType.max,
                        )
                    hid.append(ht)

                # mm2: 4 col-tiled strips into one PSUM bank; h0 pair first
                # (its relu lands first); the copy gates only this small
                # pool, keeping cv tiles relu-gated only
                ot = ops.tile([128, CH], F32, tag="o", name=f"o{g}")
                for h in range(2):
                    for i in range(2):
                        j = 2 * i + h
                        nc.tensor.matmul(
                            ot[32 * j : 32 * j + C, 0:CH],
                            w2t_t[:],
                            hid[h][:, i * CH : (i + 1) * CH],
                            start=True,
                            stop=True,
                            tile_position=(0, 32 * j),
                        )
                st = spool.tile([128, CH], BF16, tag="st", name=f"st{g}")
                if g % 5 < 3:
                    nc.scalar.activation(
                        st[:], ot[:, 0:CH], mybir.ActivationFunctionType.Copy,
                        bias=0.0, scale=1.0,
                    )
                else:
                    nc.vector.tensor_copy(st[:], ot[:, 0:CH])
                nc.gpsimd.dma_start(out=out[g, :, :], in_=st[:])

            # software-pipelined emission: mm1 runs one group ahead of the
            # drain stage so the in-order PE queue never parks on a
            # relu-gated mm2 while the next group's mm1 is ready; window
            # DMAs prefetch four groups ahead
            for g0 in range(4):
                emit_dma(g0)
            emit_mm1(0)
            for g in range(NG):
                if g + 4 < NG:
                    emit_dma(g + 4)
                if g + 1 < NG:
                    emit_mm1(g + 1)
                emit_drain(g)

    if hoist:
        _hoist_matmul_waits(nc)
    return nc


_NC_CACHE: dict = {}


def _get_nc():
    if "nc" not in _NC_CACHE:
        _NC_CACHE["nc"] = build_nc()
    return _NC_CACHE["nc"]


def host_prepare(state, W1, b1, W2):
    """Build per-core input maps. state: (H, W, C) f32."""
    xt = np.ascontiguousarray(state.transpose(2, 0, 1))  # (C, H, W)
    xtp = np.pad(xt, ((0, 0), (1, 1), (1, 1)), mode="wrap")  # (C, H+2, W+2)
    xtp_bf = xtp.astype(NPBF16)
    wtm = build_wt(W1).astype(NPBF16)  # (128, 384)
    w2t = np.ascontiguousarray(W2.T).astype(NPBF16)  # (128, 16)
    b1c = np.ascontiguousarray(b1.reshape(HID, 1)).astype(np.float32)

    in_maps = []
    for k in range(NCORES):
        r0 = k * RPC
        # xw[g, 64i + dr*16 + ch, q] = xtp[ch, r0 + 2g + dr, q]  (both halves)
        slab = np.empty((NG, 128, WP), NPBF16)
        for dr in range(4):
            rows = xtp_bf[:, r0 + dr : r0 + dr + 2 * NG : 2, :].transpose(1, 0, 2)
            slab[:, dr * C : dr * C + C] = rows
            slab[:, 64 + dr * C : 64 + dr * C + C] = rows
        in_maps.append(
            {
                "xw": np.ascontiguousarray(slab),
                "wt": wtm,
                "w2t": w2t,
                "b1": b1c,
            }
        )
    return in_maps


def assemble_out(results, b2):
    """results[k]["out"]: (NG, 128, CH) raw strips -> (H, W, C) + b2."""
    cores = []
    for k in range(NCORES):
        raw = np.asarray(results[k]["out"]).astype(np.float32)  # (NG, 128, CH)
        v = raw.reshape(NG, 2, 2, 32, CH)[:, :, :, :C, :]  # g,i,h,ch,c
        core = v.transpose(0, 1, 3, 2, 4).reshape(RPC, C, W)  # row,ch,col
        cores.append(core)
    out_t = np.concatenate(cores, axis=0)  # (H, C, W)
    return np.ascontiguousarray(
        out_t.transpose(0, 2, 1) + b2[None, None, :]
    ).astype(np.float32)


def kernel(state, W1, b1, W2, b2, **extra):
    state = np.asarray(state, np.float32)
    W1 = np.asarray(W1, np.float32)
    b1 = np.asarray(b1, np.float32)
    W2 = np.asarray(W2, np.float32)
    b2 = np.asarray(b2, np.float32)

    nc = _get_nc()
    in_maps = host_prepare(state, W1, b1, W2)
    res = run_bass_kernel_spmd(nc, in_maps, core_ids=list(range(NCORES)))
    return assemble_out(res.results, b2)


if __name__ == "__main__":
    rng = np.random.default_rng(0)
    state = rng.standard_normal((H, W, C), dtype=np.float32)
    W1 = rng.standard_normal((HID, 3 * C), dtype=np.float32) * 0.1
    b1v = rng.standard_normal(HID).astype(np.float32) * 0.1
    W2 = rng.standard_normal((C, HID), dtype=np.float32) * 0.1
    b2v = rng.standard_normal(C).astype(np.float32) * 0.1
    out = kernel(state, W1, b1v, W2, b2v)
    print(out.shape, out.dtype)


# revision 44
# speedup vs baseline: 1.0881x; 1.0001x over previous
"""Trainium2 kernel for the sobel-perception CNN cell.

Computation (per pixel, circular 3x3 stencil):
    perc = [sobel_x * x, sobel_y * x, x]            # 48 channels
    hidden = relu(W1 @ perc + b1)                   # 128 channels
    out    = W2 @ hidden + b2                       # 16 channels

The depthwise sobel convs share one 2d kernel across channels, so they
commute with the 1x1 channel-mixing conv: folding them into W1 gives
hidden = relu(sum_{dy,dx} M[dy,dx] @ x_shift(dy,dx) + b1).

Device layout (v4):
  * Window per output-row-pair: partitions [dxv(2) x dr(4) x ch(16)] = 128,
    dy folded into the partition stack; the two dxv blocks hold the row
    data at column shifts 0 / +1.
  * mm_a: one K=128 matmul covers dx = -1 (dxv0) and dx = 0 (dxv1).
  * mm_b: dx = +1, K=64.  Emitted in row-disjoint pairs — chunk A reads
    the dxv0 block at free offset +2 (partitions 0-63), chunk B reads the
    dxv1 block at offset +1 (partitions 64-127) — so consecutive mm_b's
    execute concurrently in the PE array (disjoint row groups).
  * mm2 (M=16) packs 4 chunks into one PSUM bank via 4x column tiling
    (tile_position cols 0/32/64/96) -> all 4 copied out per [128,512] op.
  * bf16 operands (PSUM stays f32); relu+bias PSUM->SBUF split between
    Scalar (activation) and Vector (tensor_scalar) engines.
  * One batched 256KB output DMA per group; host reassembles the strips.

Sharding: rows of the 1024x1024 grid split across 8 cores (128 rows each);
the host bakes the circular halos into each core's window slab, so the
device kernel needs no collectives.
"""

import sys

sys.path.insert(0, "/opt/trn_rl_repo")

import ml_dtypes
import numpy as np

import concourse.bass as bass
import concourse.mybir as mybir
from concourse.bass_utils import run_bass_kernel_spmd
from concourse.tile import TileContext

H, W, C, HID = 1024, 1024, 16, 128
NCORES = 8
RPC = H // NCORES  # rows per core
NG = RPC // 2  # groups per core (2 output rows each)
WP = W + 2  # window free length
CH = 512  # matmul free-dim chunk (one PSUM bank of fp32)

_SOBEL_X = np.array([[-1.0, 0.0, 1.0], [-2.0, 0.0, 2.0], [-1.0, 0.0, 1.0]], np.float32)
_SOBEL_Y = np.array([[-1.0, -2.0, -1.0], [0.0, 0.0, 0.0], [1.0, 2.0, 1.0]], np.float32)

F32 = mybir.dt.float32
BF16 = mybir.dt.bfloat16
NPBF16 = ml_dtypes.bfloat16


def build_a_mats(W1: np.ndarray) -> np.ndarray:
    """A[dx][o, dy*16+ch] for dx in (-1, 0, +1) -> shape (3, 128, 48)."""
    W1a, W1b, W1c = W1[:, 0:C], W1[:, C : 2 * C], W1[:, 2 * C : 3 * C]
    A = np.zeros((3, HID, 3 * C), np.float32)
    for dxi in range(3):
        for dyi in range(3):
            m = _SOBEL_X[dyi, dxi] * W1a + _SOBEL_Y[dyi, dxi] * W1b
            if dyi == 1 and dxi == 1:
                m = m + W1c
            A[dxi, :, dyi * C : (dyi + 1) * C] = m
    return A


def build_wt(W1: np.ndarray) -> np.ndarray:
    """lhsT slab [128, 3*128] for the all-K=64 layout: row-half i holds the
    three dx matrices for output row i:
    wt[64i + dr*16 + ch, dxi*128 + o] = M[dy=dr-1-i, dx=dxi-1][o, ch]
    (zero outside 0 <= dr-i <= 2)."""
    A = build_a_mats(W1)
    wt = np.zeros((128, 3 * HID), np.float32)
    for i in range(2):
        for dxi in range(3):
            for dr in range(4):
                dyi = dr - i
                if 0 <= dyi <= 2:
                    blk = A[dxi][:, dyi * C : (dyi + 1) * C]  # (128, 16)
                    p0 = 64 * i + dr * C
                    wt[p0 : p0 + C, dxi * HID : (dxi + 1) * HID] = blk.T
    return wt


def _hoist_matmul_waits(nc: bass.Bass) -> None:
    """This walrus build's instruction formats hold at most ONE sync wait,
    but Tile emits 2-3 on some instructions.  Hoist excess waits onto
    inserted same-engine NoOps (one wait each) right before the
    instruction — semantically the same blocking point on the in-order
    engine queue."""
    fixn = 0
    for fn in nc.m.functions:
        for blk in fn.blocks:
            needs_fix = any(
                inst.sync_info is not None and len(inst.sync_info.on_wait) > 1
                for inst in blk.instructions
            )
            if not needs_fix:
                continue
            out = []
            for inst in blk.instructions:
                si = inst.sync_info
                if si is not None and len(si.on_wait) > 1:
                    for w in si.on_wait:
                        nop = mybir.InstNoOp(name=f"I-mmfix-{fixn}")
                        fixn += 1
                        nop.engine = inst.engine
                        nop.sync_info = mybir.SyncInfo(on_wait=[w], on_update=[])
                        out.append(nop)
                    si.on_wait = []
                out.append(inst)
            blk.instructions = out


def build_nc(hoist: bool = True) -> bass.Bass:
    nc = bass.Bass()
    xw = nc.declare_dram_parameter("xw", [NG, 128, WP], BF16, isOutput=False)
    wt = nc.declare_dram_parameter("wt", [128, 3 * HID], BF16, isOutput=False)
    w2t = nc.declare_dram_parameter("w2t", [HID, C], BF16, isOutput=False)
    b1 = nc.declare_dram_parameter("b1", [HID, 1], F32, isOutput=False)
    # raw [g][strip j = 2i+h][ch][c] layout; host reassembles rows/cols
    out = nc.declare_dram_parameter("out", [NG, 128, CH], BF16, isOutput=True)

    with TileContext(nc) as tc:
        with (
            tc.tile_pool(name="const", bufs=1) as cpool,
            tc.tile_pool(name="xrows", bufs=8) as xpool,
            tc.tile_pool(name="hid", bufs=8) as hpool,
            tc.tile_pool(name="stage", bufs=6) as spool,
            tc.tile_pool(name="cps", bufs=3, space="PSUM") as cps,
            tc.tile_pool(name="ops", bufs=2, space="PSUM") as ops,
        ):
            # constants ride the scalar engine's HWDGE ring so the sync
            # ring streams the first window slabs without queueing behind them
            wt_t = cpool.tile([128, 3 * HID], BF16)
            nc.scalar.dma_start(out=wt_t[:], in_=wt[:])
            w2t_t = cpool.tile([HID, C], BF16)
            nc.scalar.dma_start(out=w2t_t[:], in_=w2t[:])
            b1_t = cpool.tile([HID, 1], F32)
            nc.scalar.dma_start(out=b1_t[:], in_=b1[:])

            wins: dict = {}
            cvs: dict = {}
            oreadies: dict = {}

            def emit_dma(g):
                win = xpool.tile([128, WP], BF16, tag="xrow", name=f"xw{g}")
                nc.sync.dma_start(out=win[:], in_=xw[g, :, :])
                wins[g] = win

            def emit_mm1(g):
                # all-K=64 mm1: 3 accumulating dx matmuls per chunk; the two
                # output rows (i) live in disjoint row-halves, so adjacent
                # i0/i1 matmuls run concurrently and every weight load hides
                # under the other row-half's stream
                win = wins.pop(g)
                # cv tile per h (col-halves = i): the h0 tile's chains finish
                # at the midpoint of this group's matmul burst, so its relu
                # starts early and frees the buffer for the next group in time
                cv = [
                    cps.tile([128, 2 * CH], F32, tag="cv", name=f"cv{g}_{h}")
                    for h in range(2)
                ]
                for h in range(2):
                    for dxi in range(3):
                        for i in range(2):
                            nc.tensor.matmul(
                                cv[h][:, i * CH : (i + 1) * CH],
                                wt_t[64 * i : 64 * i + 64, dxi * HID : (dxi + 1) * HID],
                                win[64 * i : 64 * i + 64, h * CH + dxi : h * CH + dxi + CH],
                                start=(dxi == 0),
                                stop=(dxi == 2),
                                tile_position=(64 * i, 0),
                            )
                cvs[g] = cv

            def emit_drain(g):
                cv = cvs.pop(g)
                # relu h0 on ACT (faster, tile ready earliest), h1 on DVE
                hid = []
                for h in range(2):
                    ht = hpool.tile([128, 2 * CH], BF16, tag="h", name=f"h{g}_{h}")
                    if h == 0:
                        nc.scalar.activation(
                            ht[:],
                            cv[h][:],
                            mybir.ActivationFunctionType.Relu,
                            bias=b1_t[:],
                            scale=1.0,
                        )
                    else:
                        nc.vector.tensor_scalar(
                            ht[:],
                            cv[h][:],
                            b1_t[:],
                            0.0,
                            mybir.AluOpType.add,
                            mybir.AluOpType.max,
                        )
                    hid.append(ht)

                # mm2: 4 col-tiled strips into one PSUM bank; h0 pair first
                # (its relu lands first); the copy gates only this small
                # pool, keeping cv tiles relu-gated only
                ot = ops.tile([128, CH], F32, tag="o", name=f"o{g}")
                for h in range(2):
                    for i in range(2):
                        j = 2 * i + h
                        nc.tensor.matmul(
                            ot[32 * j : 32 * j + C, 0:CH],
                            w2t_t[:],
                            hid[h][:, i * CH : (i + 1) * CH],
                            start=True,
                            stop=True,
                            tile_position=(0, 32 * j),
                        )
                oreadies[g] = ot

            def emit_copy(g):
                # emitted one group late so relus of g+1 sit AHEAD of this
                # copy in the ACT/DVE FIFO queues (the copy's deadline is
                # only mm2 of g+2 via the ops pool)
                ot = oreadies.pop(g)
                st = spool.tile([128, CH], BF16, tag="st", name=f"st{g}")
                if g % 5 < 3:
                    nc.scalar.activation(
                        st[:], ot[:, 0:CH], mybir.ActivationFunctionType.Copy,
                        bias=0.0, scale=1.0,
                    )
                else:
                    nc.vector.tensor_copy(st[:], ot[:, 0:CH])
                nc.gpsimd.dma_start(out=out[g, :, :], in_=st[:])

            # software-pipelined emission: mm1 runs one group ahead of the
            # drain stage (so the in-order PE queue never parks on a
            # relu-gated mm2), copies run one group behind (so relus are
            # never queued behind them); window DMAs prefetch four ahead
            for g0 in range(4):
                emit_dma(g0)
            emit_mm1(0)
            for g in range(NG):
                if g + 4 < NG:
                    emit_dma(g + 4)
                if g + 1 < NG:
                    emit_mm1(g + 1)
                emit_drain(g)
                if g >= 1:
                    emit_copy(g - 1)
            emit_copy(NG - 1)

    if hoist:
        _hoist_matmul_waits(nc)
    return nc


_NC_CACHE: dict = {}


def _get_nc():
    if "nc" not in _NC_CACHE:
        _NC_CACHE["nc"] = build_nc()
    return _NC_CACHE["nc"]


def host_prepare(state, W1, b1, W2):
    """Build per-core input maps. state: (H, W, C) f32."""
    xt = np.ascontiguousarray(state.transpose(2, 0, 1))  # (C, H, W)
    xtp = np.pad(xt, ((0, 0), (1, 1), (1, 1)), mode="wrap")  # (C, H+2, W+2)
    xtp_bf = xtp.astype(NPBF16)
    wtm = build_wt(W1).astype(NPBF16)  # (128, 384)
    w2t = np.ascontiguousarray(W2.T).astype(NPBF16)  # (128, 16)
    b1c = np.ascontiguousarray(b1.reshape(HID, 1)).astype(np.float32)

    in_maps = []
    for k in range(NCORES):
        r0 = k * RPC
        # xw[g, 64i + dr*16 + ch, q] = xtp[ch, r0 + 2g + dr, q]  (both halves)
        slab = np.empty((NG, 128, WP), NPBF16)
        for dr in range(4):
            rows = xtp_bf[:, r0 + dr : r0 + dr + 2 * NG : 2, :].transpose(1, 0, 2)
            slab[:, dr * C : dr * C + C] = rows
            slab[:, 64 + dr * C : 64 + dr * C + C] = rows
        in_maps.append(
            {
                "xw": np.ascontiguousarray(slab),
                "wt": wtm,
                "w2t": w2t,
                "b1": b1c,
            }
        )
    return in_maps


def assemble_out(results, b2):
    """results[k]["out"]: (NG, 128, CH) raw strips -> (H, W, C) + b2."""
    cores = []
    for k in range(NCORES):
        raw = np.asarray(results[k]["out"]).astype(np.float32)  # (NG, 128, CH)
        v = raw.reshape(NG, 2, 2, 32, CH)[:, :, :, :C, :]  # g,i,h,ch,c
        core = v.transpose(0, 1, 3, 2, 4).reshape(RPC, C, W)  # row,ch,col
        cores.append(core)
    out_t = np.concatenate(cores, axis=0)  # (H, C, W)
    return np.ascontiguousarray(
        out_t.transpose(0, 2, 1) + b2[None, None, :]
    ).astype(np.float32)


def kernel(state, W1, b1, W2, b2, **extra):
    state = np.asarray(state, np.float32)
    W1 = np.asarray(W1, np.float32)
    b1 = np.asarray(b1, np.float32)
    W2 = np.asarray(W2, np.float32)
    b2 = np.asarray(b2, np.float32)

    nc = _get_nc()
    in_maps = host_prepare(state, W1, b1, W2)
    res = run_bass_kernel_spmd(nc, in_maps, core_ids=list(range(NCORES)))
    return assemble_out(res.results, b2)


if __name__ == "__main__":
    rng = np.random.default_rng(0)
    state = rng.standard_normal((H, W, C), dtype=np.float32)
    W1 = rng.standard_normal((HID, 3 * C), dtype=np.float32) * 0.1
    b1v = rng.standard_normal(HID).astype(np.float32) * 0.1
    W2 = rng.standard_normal((C, HID), dtype=np.float32) * 0.1
    b2v = rng.standard_normal(C).astype(np.float32) * 0.1
    out = kernel(state, W1, b1v, W2, b2v)
    print(out.shape, out.dtype)


# revision 45
# speedup vs baseline: 1.0893x; 1.0011x over previous
"""Trainium2 kernel for the sobel-perception CNN cell.

Computation (per pixel, circular 3x3 stencil):
    perc = [sobel_x * x, sobel_y * x, x]            # 48 channels
    hidden = relu(W1 @ perc + b1)                   # 128 channels
    out    = W2 @ hidden + b2                       # 16 channels

The depthwise sobel convs share one 2d kernel across channels, so they
commute with the 1x1 channel-mixing conv: folding them into W1 gives
hidden = relu(sum_{dy,dx} M[dy,dx] @ x_shift(dy,dx) + b1).

Device layout (v4):
  * Window per output-row-pair: partitions [dxv(2) x dr(4) x ch(16)] = 128,
    dy folded into the partition stack; the two dxv blocks hold the row
    data at column shifts 0 / +1.
  * mm_a: one K=128 matmul covers dx = -1 (dxv0) and dx = 0 (dxv1).
  * mm_b: dx = +1, K=64.  Emitted in row-disjoint pairs — chunk A reads
    the dxv0 block at free offset +2 (partitions 0-63), chunk B reads the
    dxv1 block at offset +1 (partitions 64-127) — so consecutive mm_b's
    execute concurrently in the PE array (disjoint row groups).
  * mm2 (M=16) packs 4 chunks into one PSUM bank via 4x column tiling
    (tile_position cols 0/32/64/96) -> all 4 copied out per [128,512] op.
  * bf16 operands (PSUM stays f32); relu+bias PSUM->SBUF split between
    Scalar (activation) and Vector (tensor_scalar) engines.
  * One batched 256KB output DMA per group; host reassembles the strips.

Sharding: rows of the 1024x1024 grid split across 8 cores (128 rows each);
the host bakes the circular halos into each core's window slab, so the
device kernel needs no collectives.
"""

import sys

sys.path.insert(0, "/opt/trn_rl_repo")

import ml_dtypes
import numpy as np

import concourse.bass as bass
import concourse.mybir as mybir
from concourse.bass_utils import run_bass_kernel_spmd
from concourse.tile import TileContext

H, W, C, HID = 1024, 1024, 16, 128
NCORES = 8
RPC = H // NCORES  # rows per core
NG = RPC // 2  # groups per core (2 output rows each)
WP = W + 2  # window free length
CH = 512  # matmul free-dim chunk (one PSUM bank of fp32)

_SOBEL_X = np.array([[-1.0, 0.0, 1.0], [-2.0, 0.0, 2.0], [-1.0, 0.0, 1.0]], np.float32)
_SOBEL_Y = np.array([[-1.0, -2.0, -1.0], [0.0, 0.0, 0.0], [1.0, 2.0, 1.0]], np.float32)

F32 = mybir.dt.float32
BF16 = mybir.dt.bfloat16
NPBF16 = ml_dtypes.bfloat16


def build_a_mats(W1: np.ndarray) -> np.ndarray:
    """A[dx][o, dy*16+ch] for dx in (-1, 0, +1) -> shape (3, 128, 48)."""
    W1a, W1b, W1c = W1[:, 0:C], W1[:, C : 2 * C], W1[:, 2 * C : 3 * C]
    A = np.zeros((3, HID, 3 * C), np.float32)
    for dxi in range(3):
        for dyi in range(3):
            m = _SOBEL_X[dyi, dxi] * W1a + _SOBEL_Y[dyi, dxi] * W1b
            if dyi == 1 and dxi == 1:
                m = m + W1c
            A[dxi, :, dyi * C : (dyi + 1) * C] = m
    return A


def build_wt(W1: np.ndarray) -> np.ndarray:
    """lhsT slab [128, 3*128] for the all-K=64 layout: row-half i holds the
    three dx matrices for output row i:
    wt[64i + dr*16 + ch, dxi*128 + o] = M[dy=dr-1-i, dx=dxi-1][o, ch]
    (zero outside 0 <= dr-i <= 2)."""
    A = build_a_mats(W1)
    wt = np.zeros((128, 3 * HID), np.float32)
    for i in range(2):
        for dxi in range(3):
            for dr in range(4):
                dyi = dr - i
                if 0 <= dyi <= 2:
                    blk = A[dxi][:, dyi * C : (dyi + 1) * C]  # (128, 16)
                    p0 = 64 * i + dr * C
                    wt[p0 : p0 + C, dxi * HID : (dxi + 1) * HID] = blk.T
    return wt


def _hoist_matmul_waits(nc: bass.Bass) -> None:
    """This walrus build's instruction formats hold at most ONE sync wait,
    but Tile emits 2-3 on some instructions.  Hoist excess waits onto
    inserted same-engine NoOps (one wait each) right before the
    instruction — semantically the same blocking point on the in-order
    engine queue."""
    fixn = 0
    for fn in nc.m.functions:
        for blk in fn.blocks:
            needs_fix = any(
                inst.sync_info is not None and len(inst.sync_info.on_wait) > 1
                for inst in blk.instructions
            )
            if not needs_fix:
                continue
            out = []
            for inst in blk.instructions:
                si = inst.sync_info
                if si is not None and len(si.on_wait) > 1:
                    for w in si.on_wait:
                        nop = mybir.InstNoOp(name=f"I-mmfix-{fixn}")
                        fixn += 1
                        nop.engine = inst.engine
                        nop.sync_info = mybir.SyncInfo(on_wait=[w], on_update=[])
                        out.append(nop)
                    si.on_wait = []
                out.append(inst)
            blk.instructions = out


def build_nc(hoist: bool = True) -> bass.Bass:
    nc = bass.Bass()
    xw = nc.declare_dram_parameter("xw", [NG, 128, WP], BF16, isOutput=False)
    wt = nc.declare_dram_parameter("wt", [128, 3 * HID], BF16, isOutput=False)
    w2t = nc.declare_dram_parameter("w2t", [HID, C], BF16, isOutput=False)
    b1 = nc.declare_dram_parameter("b1", [HID, 1], F32, isOutput=False)
    # raw [g][strip j = 2i+h][ch][c] layout; host reassembles rows/cols
    out = nc.declare_dram_parameter("out", [NG, 128, CH], BF16, isOutput=True)

    with TileContext(nc) as tc:
        with (
            tc.tile_pool(name="const", bufs=1) as cpool,
            tc.tile_pool(name="xrows", bufs=8) as xpool,
            tc.tile_pool(name="hid", bufs=8) as hpool,
            tc.tile_pool(name="stage", bufs=6) as spool,
            tc.tile_pool(name="cps", bufs=3, space="PSUM") as cps,
            tc.tile_pool(name="ops", bufs=2, space="PSUM") as ops,
        ):
            # constants ride the scalar engine's HWDGE ring so the sync
            # ring streams the first window slabs without queueing behind them
            wt_t = cpool.tile([128, 3 * HID], BF16)
            nc.scalar.dma_start(out=wt_t[:], in_=wt[:])
            w2t_t = cpool.tile([HID, C], BF16)
            nc.scalar.dma_start(out=w2t_t[:], in_=w2t[:])
            b1_t = cpool.tile([HID, 1], F32)
            nc.scalar.dma_start(out=b1_t[:], in_=b1[:])

            wins: dict = {}
            cvs: dict = {}
            oreadies: dict = {}

            def emit_dma(g):
                win = xpool.tile([128, WP], BF16, tag="xrow", name=f"xw{g}")
                nc.sync.dma_start(out=win[:], in_=xw[g, :, :])
                wins[g] = win

            def emit_mm1(g):
                # all-K=64 mm1: 3 accumulating dx matmuls per chunk; the two
                # output rows (i) live in disjoint row-halves, so adjacent
                # i0/i1 matmuls run concurrently and every weight load hides
                # under the other row-half's stream
                win = wins.pop(g)
                # cv tile per h (col-halves = i): the h0 tile's chains finish
                # at the midpoint of this group's matmul burst, so its relu
                # starts early and frees the buffer for the next group in time
                cv = [
                    cps.tile([128, 2 * CH], F32, tag="cv", name=f"cv{g}_{h}")
                    for h in range(2)
                ]
                for h in range(2):
                    for dxi in range(3):
                        for i in range(2):
                            nc.tensor.matmul(
                                cv[h][:, i * CH : (i + 1) * CH],
                                wt_t[64 * i : 64 * i + 64, dxi * HID : (dxi + 1) * HID],
                                win[64 * i : 64 * i + 64, h * CH + dxi : h * CH + dxi + CH],
                                start=(dxi == 0),
                                stop=(dxi == 2),
                                tile_position=(64 * i, 0),
                            )
                cvs[g] = cv

            def emit_drain(g):
                cv = cvs.pop(g)
                # h0's tile is ready mid-burst (slack for the slower DVE);
                # h1's relu is the late, mm2-gating one -> faster ACT
                hid = []
                for h in range(2):
                    ht = hpool.tile([128, 2 * CH], BF16, tag="h", name=f"h{g}_{h}")
                    if h == 1:
                        nc.scalar.activation(
                            ht[:],
                            cv[h][:],
                            mybir.ActivationFunctionType.Relu,
                            bias=b1_t[:],
                            scale=1.0,
                        )
                    else:
                        nc.vector.tensor_scalar(
                            ht[:],
                            cv[h][:],
                            b1_t[:],
                            0.0,
                            mybir.AluOpType.add,
                            mybir.AluOpType.max,
                        )
                    hid.append(ht)

                # mm2: 4 col-tiled strips into one PSUM bank; h0 pair first
                # (its relu lands first); the copy gates only this small
                # pool, keeping cv tiles relu-gated only
                ot = ops.tile([128, CH], F32, tag="o", name=f"o{g}")
                for h in range(2):
                    for i in range(2):
                        j = 2 * i + h
                        nc.tensor.matmul(
                            ot[32 * j : 32 * j + C, 0:CH],
                            w2t_t[:],
                            hid[h][:, i * CH : (i + 1) * CH],
                            start=True,
                            stop=True,
                            tile_position=(0, 32 * j),
                        )
                oreadies[g] = ot

            def emit_copy(g):
                # emitted one group late so relus of g+1 sit AHEAD of this
                # copy in the ACT/DVE FIFO queues (the copy's deadline is
                # only mm2 of g+2 via the ops pool)
                ot = oreadies.pop(g)
                st = spool.tile([128, CH], BF16, tag="st", name=f"st{g}")
                if g % 5 < 3:
                    nc.scalar.activation(
                        st[:], ot[:, 0:CH], mybir.ActivationFunctionType.Copy,
                        bias=0.0, scale=1.0,
                    )
                else:
                    nc.vector.tensor_copy(st[:], ot[:, 0:CH])
                nc.gpsimd.dma_start(out=out[g, :, :], in_=st[:])

            # software-pipelined emission: mm1 runs one group ahead of the
            # drain stage (so the in-order PE queue never parks on a
            # relu-gated mm2), copies run one group behind (so relus are
            # never queued behind them); window DMAs prefetch four ahead
            for g0 in range(4):
                emit_dma(g0)
            emit_mm1(0)
            for g in range(NG):
                if g + 4 < NG:
                    emit_dma(g + 4)
                if g + 1 < NG:
                    emit_mm1(g + 1)
                emit_drain(g)
                if g >= 1:
                    emit_copy(g - 1)
            emit_copy(NG - 1)

    if hoist:
        _hoist_matmul_waits(nc)
    return nc


_NC_CACHE: dict = {}


def _get_nc():
    if "nc" not in _NC_CACHE:
        _NC_CACHE["nc"] = build_nc()
    return _NC_CACHE["nc"]


def host_prepare(state, W1, b1, W2):
    """Build per-core input maps. state: (H, W, C) f32."""
    xt = np.ascontiguousarray(state.transpose(2, 0, 1))  # (C, H, W)
    xtp = np.pad(xt, ((0, 0), (1, 1), (1, 1)), mode="wrap")  # (C, H+2, W+2)
    xtp_bf = xtp.astype(NPBF16)
    wtm = build_wt(W1).astype(NPBF16)  # (128, 384)
    w2t = np.ascontiguousarray(W2.T).astype(NPBF16)  # (128, 16)
    b1c = np.ascontiguousarray(b1.reshape(HID, 1)).astype(np.float32)

    in_maps = []
    for k in range(NCORES):
        r0 = k * RPC
        # xw[g, 64i + dr*16 + ch, q] = xtp[ch, r0 + 2g + dr, q]  (both halves)
        slab = np.empty((NG, 128, WP), NPBF16)
        for dr in range(4):
            rows = xtp_bf[:, r0 + dr : r0 + dr + 2 * NG : 2, :].transpose(1, 0, 2)
            slab[:, dr * C : dr * C + C] = rows
            slab[:, 64 + dr * C : 64 + dr * C + C] = rows
        in_maps.append(
            {
                "xw": np.ascontiguousarray(slab),
                "wt": wtm,
                "w2t": w2t,
                "b1": b1c,
            }
        )
    return in_maps


def assemble_out(results, b2):
    """results[k]["out"]: (NG, 128, CH) raw strips -> (H, W, C) + b2."""
    cores = []
    for k in range(NCORES):
        raw = np.asarray(results[k]["out"]).astype(np.float32)  # (NG, 128, CH)
        v = raw.reshape(NG, 2, 2, 32, CH)[:, :, :, :C, :]  # g,i,h,ch,c
        core = v.transpose(0, 1, 3, 2, 4).reshape(RPC, C, W)  # row,ch,col
        cores.append(core)
    out_t = np.concatenate(cores, axis=0)  # (H, C, W)
    return np.ascontiguousarray(
        out_t.transpose(0, 2, 1) + b2[None, None, :]
    ).astype(np.float32)


def kernel(state, W1, b1, W2, b2, **extra):
    state = np.asarray(state, np.float32)
    W1 = np.asarray(W1, np.float32)
    b1 = np.asarray(b1, np.float32)
    W2 = np.asarray(W2, np.float32)
    b2 = np.asarray(b2, np.float32)

    nc = _get_nc()
    in_maps = host_prepare(state, W1, b1, W2)
    res = run_bass_kernel_spmd(nc, in_maps, core_ids=list(range(NCORES)))
    return assemble_out(res.results, b2)


if __name__ == "__main__":
    rng = np.random.default_rng(0)
    state = rng.standard_normal((H, W, C), dtype=np.float32)
    W1 = rng.standard_normal((HID, 3 * C), dtype=np.float32) * 0.1
    b1v = rng.standard_normal(HID).astype(np.float32) * 0.1
    W2 = rng.standard_normal((C, HID), dtype=np.float32) * 0.1
    b2v = rng.standard_normal(C).astype(np.float32) * 0.1
    out = kernel(state, W1, b1v, W2, b2v)
    print(out.shape, out.dtype)
